# revision 1
# baseline (speedup 1.0000x reference)
"""Trainium2 Bass kernel for nn_Conv_LI (leaky-integrator + 5x5 'same' conv).

Math: with the reference constants, DT*TAU_MEM_INV = 1.0, so the LI cell
collapses to
    vs[t] = i_t,   i_{t+1} = (i_t - 0.2*i_t) + x_t,   i_0 = 0
(an exponential moving accumulation over time), followed by a per-timestep
5x5 cross-correlation with 'same' zero padding.

Distribution: H is sharded across the 8 cores (64 output rows each). Each
core receives its 64 rows plus a 2-row halo on each side (zero-padded at the
global edges), so no inter-core communication is needed.

Per-core pipeline (all 8 cores run the same program, SPMD):
  - x arrives host-side time-shifted by one (vs[t] needs x[t-1]) and
    zero-padded to [T, 68, 516] (h halo + w halo).
  - DMA a window of TW=16 timesteps into SBUF, layout [68 part, t, w].
  - EMA on VectorE: one scalar_tensor_tensor per timestep:
        vs[s] = (vs[s-1] * 0.8) + x[s]
  - 5x5 conv on TensorE as 5 PSUM-accumulated banded matmuls (contraction
    over the h-halo partitions; the dx shifts are free-dim AP offsets).
    Two timesteps are column-packed per PSUM tile (tile_position (0,0) and
    (0,64)) so the full 128 PE columns are used.
  - ScalarE copies PSUM -> SBUF, then DMA out.
"""

import numpy as np

T_FULL, H_FULL, W_FULL = 256, 512, 512
N_CORES = 8
HC = H_FULL // N_CORES  # 64 output rows per core
HP = HC + 4             # 68 partition rows incl 2+2 halo
WP = W_FULL + 4         # 516 padded width
TW = 16                 # timesteps per window
DECAY = 0.8

_PROG_CACHE = {}


def _build_program(t_total):
    import concourse.bacc as bacc
    import concourse.mybir as mybir
    import concourse.tile as tile

    f32 = mybir.dt.float32
    f32r = mybir.dt.float32r
    mult = mybir.AluOpType.mult
    add = mybir.AluOpType.add

    assert t_total % TW == 0
    nwin = t_total // TW

    nc = bacc.Bacc(None, target_bir_lowering=False)
    x = nc.dram_tensor("x", [t_total, HP, WP], f32, kind="ExternalInput")
    lw_d = nc.dram_tensor("lw", [HP, 5 * HC], f32r, kind="ExternalInput")
    out = nc.dram_tensor("out", [t_total, HC, W_FULL], f32, kind="ExternalOutput")

    with tile.TileContext(nc) as tc:
        with (
            tc.tile_pool(name="const", bufs=1) as cpool,
            tc.tile_pool(name="xw", bufs=2) as xpool,
            tc.tile_pool(name="vs", bufs=2) as vpool,
            tc.tile_pool(name="ob", bufs=4) as opool,
            tc.tile_pool(name="ps", bufs=4, space="PSUM") as ppool,
        ):
            lw = cpool.tile([HP, 5 * HC], f32r)
            nc.sync.dma_start(out=lw[:HP, :], in_=lw_d[:, :])
            zt = cpool.tile([HP, WP], f32)
            nc.vector.memset(zt[:HP, :], 0.0)

            prev = None
            for win in range(nwin):
                t0 = win * TW
                xw = xpool.tile([HP, TW * WP], f32)
                nc.sync.dma_start(
                    out=xw[:HP, :].rearrange("h (t w) -> h t w", t=TW),
                    in_=x[t0 : t0 + TW].rearrange("t h w -> h t w"),
                )
                vs = vpool.tile([HP, TW * WP], f32r)
                # Wait-absorbing fence: scalar_tensor_tensor's ISA struct only
                # supports a single sync wait, so soak up the DMA-completion
                # and vs-slot-reuse waits on a cheap copy first.
                nc.vector.tensor_copy(out=vs[:HP, 0:4], in_=xw[:HP, 0:4])
                for s in range(TW):
                    cur = vs[:HP, s * WP : (s + 1) * WP]
                    p = zt[:HP, :] if prev is None else prev
                    nc.vector.scalar_tensor_tensor(
                        out=cur,
                        in0=p,
                        scalar=DECAY,
                        in1=xw[:HP, s * WP : (s + 1) * WP],
                        op0=mult,
                        op1=add,
                    )
                    prev = cur
                for pr in range(TW // 2):
                    sa = 2 * pr
                    # two timesteps share one 2-bank PSUM tile (free halves)
                    ps = ppool.tile([HC, 2 * W_FULL], f32)
                    for half in range(2):
                        s = sa + half
                        for dx in range(5):
                            lwx = lw[:HP, dx * HC : (dx + 1) * HC]
                            nc.tensor.matmul(
                                ps[0:HC, half * W_FULL : (half + 1) * W_FULL],
                                lwx,
                                vs[:HP, s * WP + dx : s * WP + dx + W_FULL],
                                start=(dx == 0),
                                stop=(dx == 4),
                            )
                    ob = opool.tile([HC, 2 * W_FULL], f32)
                    nc.scalar.copy(out=ob[0:HC, :], in_=ps[0:HC, :])
                    nc.sync.dma_start(
                        out=out[t0 + sa : t0 + sa + 2].rearrange("t h w -> h t w"),
                        in_=ob[0:HC, :].rearrange("h (t w) -> h t w", t=2),
                    )
    nc.finalize()
    return nc


def _get_program(t_total):
    if t_total not in _PROG_CACHE:
        _PROG_CACHE[t_total] = _build_program(t_total)
    return _PROG_CACHE[t_total]


def _host_prep(x, k, t_total):
    """Build per-core shifted+padded inputs and the banded lhsT matrices."""
    x = np.asarray(x, dtype=np.float32)
    k = np.asarray(k, dtype=np.float32)
    # time-shift by one (vs[t] = EMA consumes x[t-1]) and zero-pad h/w by 2
    xs = np.zeros((t_total, H_FULL + 4, W_FULL + 4), np.float32)
    xs[1:, 2 : H_FULL + 2, 2 : W_FULL + 2] = x[: t_total - 1, 0]
    # banded conv matrices: lhsT[p, dx, j] = k[p - j, dx] for p - j in [0, 5)
    lwh = np.zeros((HP, 5, HC), np.float32)
    j = np.arange(HC)
    for dy in range(5):
        for dx in range(5):
            lwh[j + dy, dx, j] = k[dy, dx]
    lwh = np.ascontiguousarray(lwh.reshape(HP, 5 * HC))
    in_maps = []
    for c in range(N_CORES):
        xc = np.ascontiguousarray(xs[:, c * HC : c * HC + HP, :])
        in_maps.append({"x": xc, "lw": lwh})
    return in_maps


def kernel(x, kernel):
    from concourse.bass_utils import run_bass_kernel_spmd

    t_total = x.shape[0]
    in_maps = _host_prep(x, kernel, t_total)
    nc = _get_program(t_total)
    res = run_bass_kernel_spmd(nc, in_maps, list(range(N_CORES)))
    out = np.empty((t_total, 1, H_FULL, W_FULL), np.float32)
    for c in range(N_CORES):
        out[:, 0, c * HC : (c + 1) * HC, :] = np.asarray(res.results[c]["out"])
    return out



# revision 4
# speedup vs baseline: 2.2778x; 2.2778x over previous
"""Trainium2 Bass kernel for nn_Conv_LI (leaky-integrator + 5x5 'same' conv).

Math: with the reference constants, DT*TAU_MEM_INV = 1.0, so the LI cell
collapses to
    vs[t] = i_t,   i_{t+1} = (i_t - 0.2*i_t) + x_t,   i_0 = 0
(an exponential moving accumulation over time), followed by a per-timestep
5x5 cross-correlation with 'same' zero padding.

Distribution: H is sharded across the 8 cores (64 output rows each). Each
core receives its 64 rows plus a 2-row halo on each side (zero-padded at the
global edges), so no inter-core communication is needed.

Per-core pipeline (all 8 cores run the same program, SPMD):
  - x arrives host-side as bf16, time-shifted by one (vs[t] needs x[t-1]),
    zero-padded to [68, 516] spatially, and laid out [h, t, w] so each
    window DMA reads one contiguous 33 KB run per partition.
  - EMA on VectorE in bf16: one scalar_tensor_tensor per timestep:
        vs[s] = (vs[s-1] * 0.8) + x[s]
  - 5x5 conv on TensorE as 5 PSUM-accumulated banded bf16 matmuls
    (contraction over the h-halo partitions; dx shifts are free-dim AP
    offsets). Two timesteps of one pair go to the two column halves of a
    single [128, 512] PSUM bank via tile_position (0,0)/(0,64), so both
    matmuls run concurrently on the PE array.
  - ScalarE copies PSUM -> SBUF with bf16 downcast; output DMA rides the
    scalar HWDGE ring (input rides the sync ring) and the host upcasts.
"""

import numpy as np

T_FULL, H_FULL, W_FULL = 256, 512, 512
N_CORES = 8
HC = H_FULL // N_CORES  # 64 output rows per core
HP = HC + 4             # 68 partition rows incl 2+2 halo
WP = W_FULL + 4         # 516 padded width
TW = 32                 # timesteps per window
PBLK = 4                # psum pairs per eviction block (4 pairs = 8 steps)
DECAY = 0.8

_PROG_CACHE = {}


def _build_program(t_total):
    import concourse.bacc as bacc
    import concourse.mybir as mybir
    import concourse.tile as tile

    f32 = mybir.dt.float32
    bf16 = mybir.dt.bfloat16
    mult = mybir.AluOpType.mult
    add = mybir.AluOpType.add

    assert t_total % (2 * PBLK * TW // TW) == 0 and t_total % TW == 0
    nwin = t_total // TW
    nblk = TW // (2 * PBLK)  # eviction blocks per window

    nc = bacc.Bacc(None, target_bir_lowering=False)
    x = nc.dram_tensor("x", [HP, t_total, WP], bf16, kind="ExternalInput")
    lw_d = nc.dram_tensor("lw", [HP, 5 * HC], bf16, kind="ExternalInput")
    # out layout [s, h, pair, w]: partition line (s,h) writes 4 KB runs
    out = nc.dram_tensor(
        "out", [2, HC, t_total // 2, W_FULL], bf16, kind="ExternalOutput"
    )

    with tile.TileContext(nc) as tc:
        with (
            tc.tile_pool(name="const", bufs=1) as cpool,
            tc.tile_pool(name="xw", bufs=2) as xpool,
            tc.tile_pool(name="vs", bufs=2) as vpool,
            tc.tile_pool(name="ob", bufs=3) as opool,
            tc.tile_pool(name="ps", bufs=2, space="PSUM") as ppool,
        ):
            lw = cpool.tile([HP, 5 * HC], bf16)
            nc.sync.dma_start(out=lw[:HP, :], in_=lw_d[:, :])
            zt = cpool.tile([HP, WP], bf16)
            nc.vector.memset(zt[:HP, :], 0.0)

            prev = None
            for win in range(nwin):
                t0 = win * TW
                xw = xpool.tile([HP, TW * WP], bf16)
                nc.sync.dma_start(
                    out=xw[:HP, :].rearrange("h (t w) -> h t w", t=TW),
                    in_=x[:, t0 : t0 + TW, :],
                )
                vs = vpool.tile([HP, TW * WP], bf16)
                # Wait-absorbing fence: scalar_tensor_tensor's ISA struct only
                # supports a single sync wait, so soak up the DMA-completion
                # and vs-slot-reuse waits on a cheap copy first.
                nc.vector.tensor_copy(out=vs[:HP, 0:4], in_=xw[:HP, 0:4])
                for s in range(TW):
                    cur = vs[:HP, s * WP : (s + 1) * WP]
                    p = zt[:HP, :] if prev is None else prev
                    nc.vector.scalar_tensor_tensor(
                        out=cur,
                        in0=p,
                        scalar=DECAY,
                        in1=xw[:HP, s * WP : (s + 1) * WP],
                        op0=mult,
                        op1=add,
                    )
                    prev = cur
                for pb in range(nblk):
                    pss = [
                        ppool.tile([2 * HC, W_FULL], f32, name=f"ps{i}")
                        for i in range(PBLK)
                    ]
                    for dx in range(5):
                        lwx = lw[:HP, dx * HC : (dx + 1) * HC]
                        for pr in range(PBLK):
                            for s2 in range(2):
                                tl = (pb * PBLK + pr) * 2 + s2
                                nc.tensor.matmul(
                                    pss[pr][s2 * HC : (s2 + 1) * HC, :],
                                    lwx,
                                    vs[:HP, tl * WP + dx : tl * WP + dx + W_FULL],
                                    start=(dx == 0),
                                    stop=(dx == 4),
                                )
                    ob = opool.tile([2 * HC, PBLK * W_FULL], bf16)
                    for pr in range(PBLK):
                        nc.scalar.copy(
                            out=ob[:, pr * W_FULL : (pr + 1) * W_FULL],
                            in_=pss[pr][:, :],
                        )
                    gpb = win * nblk + pb
                    nc.scalar.dma_start(
                        out=out[:, :, gpb * PBLK : (gpb + 1) * PBLK, :].rearrange(
                            "s h p w -> (s h) p w"
                        ),
                        in_=ob[:, :].rearrange("q (p w) -> q p w", p=PBLK),
                    )
    nc.finalize()
    return nc


def _get_program(t_total):
    if t_total not in _PROG_CACHE:
        _PROG_CACHE[t_total] = _build_program(t_total)
    return _PROG_CACHE[t_total]


def _host_prep(x, k, t_total):
    """Build per-core shifted+padded bf16 inputs and banded lhsT matrices."""
    import ml_dtypes

    x = np.asarray(x, dtype=np.float32)
    k = np.asarray(k, dtype=np.float32)
    # time-shift by one (vs[t] = EMA consumes x[t-1]), zero-pad h/w by 2,
    # cast bf16, and transpose to [h, t, w] for contiguous window DMAs
    xs = np.zeros((t_total, H_FULL + 4, W_FULL + 4), ml_dtypes.bfloat16)
    xs[1:, 2 : H_FULL + 2, 2 : W_FULL + 2] = x[: t_total - 1, 0].astype(
        ml_dtypes.bfloat16
    )
    # banded conv matrices: lhsT[p, dx, j] = k[p - j, dx] for p - j in [0, 5)
    lwh = np.zeros((HP, 5, HC), np.float32)
    j = np.arange(HC)
    for dy in range(5):
        for dx in range(5):
            lwh[j + dy, dx, j] = k[dy, dx]
    lwh = np.ascontiguousarray(
        lwh.reshape(HP, 5 * HC).astype(ml_dtypes.bfloat16)
    )
    in_maps = []
    for c in range(N_CORES):
        xc = np.ascontiguousarray(
            xs[:, c * HC : c * HC + HP, :].transpose(1, 0, 2)
        )
        in_maps.append({"x": xc, "lw": lwh})
    return in_maps


def kernel(x, kernel):
    from concourse.bass_utils import run_bass_kernel_spmd

    t_total = x.shape[0]
    in_maps = _host_prep(x, kernel, t_total)
    nc = _get_program(t_total)
    res = run_bass_kernel_spmd(nc, in_maps, list(range(N_CORES)))
    out = np.empty((t_total, 1, H_FULL, W_FULL), np.float32)
    for c in range(N_CORES):
        # o is [s, h, pair, w]; t = 2*pair + s
        o = np.asarray(res.results[c]["out"]).astype(np.float32)
        out[:, 0, c * HC : (c + 1) * HC, :] = o.transpose(2, 0, 1, 3).reshape(
            t_total, HC, W_FULL
        )
    return out


# revision 5
# speedup vs baseline: 2.6006x; 1.1417x over previous
"""Trainium2 Bass kernel for nn_Conv_LI (leaky-integrator + 5x5 'same' conv).

Math: with the reference constants, DT*TAU_MEM_INV = 1.0, so the LI cell
collapses to
    vs[t] = i_t,   i_{t+1} = (i_t - 0.2*i_t) + x_t,   i_0 = 0
(an exponential moving accumulation over time), followed by a per-timestep
5x5 cross-correlation with 'same' zero padding.

Distribution: H is sharded across the 8 cores (64 output rows each). Each
core receives its 64 rows plus a 2-row halo on each side (zero-padded at the
global edges), so no inter-core communication is needed.

Per-core pipeline (all 8 cores run the same program, SPMD):
  - x arrives host-side as bf16, time-shifted by one (vs[t] needs x[t-1]),
    zero-padded to [68, 516] spatially, and laid out [h, t, w] so each
    window DMA reads one contiguous 33 KB run per partition.
  - EMA on VectorE in bf16: one scalar_tensor_tensor per timestep:
        vs[s] = (vs[s-1] * 0.8) + x[s]
  - 5x5 conv on TensorE as 5 PSUM-accumulated banded bf16 matmuls
    (contraction over the h-halo partitions; dx shifts are free-dim AP
    offsets). Two timesteps of one pair go to the two column halves of a
    single [128, 512] PSUM bank via tile_position (0,0)/(0,64), so both
    matmuls run concurrently on the PE array.
  - ScalarE copies PSUM -> SBUF with bf16 downcast; output DMA rides the
    scalar HWDGE ring (input rides the sync ring) and the host upcasts.
"""

import numpy as np

T_FULL, H_FULL, W_FULL = 256, 512, 512
N_CORES = 8
HC = H_FULL // N_CORES  # 64 output rows per core
HP = HC + 4             # 68 partition rows incl 2+2 halo
WP = W_FULL + 4         # 516 padded width
TW = 32                 # timesteps per window
PBLK = 4                # psum pairs per eviction block (4 pairs = 8 steps)
DECAY = 0.8

_PROG_CACHE = {}


def _build_program(t_total):
    import concourse.bacc as bacc
    import concourse.mybir as mybir
    import concourse.tile as tile

    f32 = mybir.dt.float32
    bf16 = mybir.dt.bfloat16
    mult = mybir.AluOpType.mult
    add = mybir.AluOpType.add

    assert t_total % (2 * PBLK * TW // TW) == 0 and t_total % TW == 0
    nwin = t_total // TW
    nblk = TW // (2 * PBLK)  # eviction blocks per window

    nc = bacc.Bacc(None, target_bir_lowering=False)
    x = nc.dram_tensor("x", [HP, t_total, WP], bf16, kind="ExternalInput")
    lw_d = nc.dram_tensor("lw", [HP, 5 * HC], bf16, kind="ExternalInput")
    # out layout [s, h, pair, w]: partition line (s,h) writes 4 KB runs
    out = nc.dram_tensor(
        "out", [2, HC, t_total // 2, W_FULL], bf16, kind="ExternalOutput"
    )

    with tile.TileContext(nc) as tc:
        with (
            tc.tile_pool(name="const", bufs=1) as cpool,
            tc.tile_pool(name="xw", bufs=2) as xpool,
            tc.tile_pool(name="vs", bufs=2) as vpool,
            tc.tile_pool(name="ob", bufs=3) as opool,
            tc.tile_pool(name="ps", bufs=2, space="PSUM") as ppool,
        ):
            lw = cpool.tile([HP, 5 * HC], bf16)
            nc.sync.dma_start(out=lw[:HP, :], in_=lw_d[:, :])
            zt = cpool.tile([HP, WP], bf16)
            nc.vector.memset(zt[:HP, :], 0.0)

            prev = None
            for win in range(nwin):
                t0 = win * TW
                xw = xpool.tile([HP, TW * WP], bf16)
                # Split the window load into 4-timestep sub-DMAs: 4 KB
                # descriptors spread across all 16 SDMA engines (a single
                # 33 KB/partition transfer lands on only 4), and the EMA can
                # start as soon as the first slice arrives.
                for q in range(0, TW, 4):
                    nc.sync.dma_start(
                        out=xw[:HP, q * WP : (q + 4) * WP].rearrange(
                            "h (t w) -> h t w", t=4
                        ),
                        in_=x[:, t0 + q : t0 + q + 4, :],
                    )
                vs = vpool.tile([HP, TW * WP], bf16)
                # Wait-absorbing fence: scalar_tensor_tensor's ISA struct only
                # supports a single sync wait, so soak up the DMA-completion
                # and vs-slot-reuse waits on a cheap copy first.
                nc.vector.tensor_copy(out=vs[:HP, 0:4], in_=xw[:HP, 0:4])
                for s in range(TW):
                    cur = vs[:HP, s * WP : (s + 1) * WP]
                    p = zt[:HP, :] if prev is None else prev
                    nc.vector.scalar_tensor_tensor(
                        out=cur,
                        in0=p,
                        scalar=DECAY,
                        in1=xw[:HP, s * WP : (s + 1) * WP],
                        op0=mult,
                        op1=add,
                    )
                    prev = cur
                for pb in range(nblk):
                    pss = [
                        ppool.tile([2 * HC, W_FULL], f32, name=f"ps{i}")
                        for i in range(PBLK)
                    ]
                    for dx in range(5):
                        lwx = lw[:HP, dx * HC : (dx + 1) * HC]
                        for pr in range(PBLK):
                            for s2 in range(2):
                                tl = (pb * PBLK + pr) * 2 + s2
                                nc.tensor.matmul(
                                    pss[pr][s2 * HC : (s2 + 1) * HC, :],
                                    lwx,
                                    vs[:HP, tl * WP + dx : tl * WP + dx + W_FULL],
                                    start=(dx == 0),
                                    stop=(dx == 4),
                                )
                    ob = opool.tile([2 * HC, PBLK * W_FULL], bf16)
                    for pr in range(PBLK):
                        nc.scalar.copy(
                            out=ob[:, pr * W_FULL : (pr + 1) * W_FULL],
                            in_=pss[pr][:, :],
                        )
                    gpb = win * nblk + pb
                    nc.scalar.dma_start(
                        out=out[:, :, gpb * PBLK : (gpb + 1) * PBLK, :].rearrange(
                            "s h p w -> (s h) p w"
                        ),
                        in_=ob[:, :].rearrange("q (p w) -> q p w", p=PBLK),
                    )
    nc.finalize()
    return nc


def _get_program(t_total):
    if t_total not in _PROG_CACHE:
        _PROG_CACHE[t_total] = _build_program(t_total)
    return _PROG_CACHE[t_total]


def _host_prep(x, k, t_total):
    """Build per-core shifted+padded bf16 inputs and banded lhsT matrices."""
    import ml_dtypes

    x = np.asarray(x, dtype=np.float32)
    k = np.asarray(k, dtype=np.float32)
    # time-shift by one (vs[t] = EMA consumes x[t-1]), zero-pad h/w by 2,
    # cast bf16, and transpose to [h, t, w] for contiguous window DMAs
    xs = np.zeros((t_total, H_FULL + 4, W_FULL + 4), ml_dtypes.bfloat16)
    xs[1:, 2 : H_FULL + 2, 2 : W_FULL + 2] = x[: t_total - 1, 0].astype(
        ml_dtypes.bfloat16
    )
    # banded conv matrices: lhsT[p, dx, j] = k[p - j, dx] for p - j in [0, 5)
    lwh = np.zeros((HP, 5, HC), np.float32)
    j = np.arange(HC)
    for dy in range(5):
        for dx in range(5):
            lwh[j + dy, dx, j] = k[dy, dx]
    lwh = np.ascontiguousarray(
        lwh.reshape(HP, 5 * HC).astype(ml_dtypes.bfloat16)
    )
    in_maps = []
    for c in range(N_CORES):
        xc = np.ascontiguousarray(
            xs[:, c * HC : c * HC + HP, :].transpose(1, 0, 2)
        )
        in_maps.append({"x": xc, "lw": lwh})
    return in_maps


def kernel(x, kernel):
    from concourse.bass_utils import run_bass_kernel_spmd

    t_total = x.shape[0]
    in_maps = _host_prep(x, kernel, t_total)
    nc = _get_program(t_total)
    res = run_bass_kernel_spmd(nc, in_maps, list(range(N_CORES)))
    out = np.empty((t_total, 1, H_FULL, W_FULL), np.float32)
    for c in range(N_CORES):
        # o is [s, h, pair, w]; t = 2*pair + s
        o = np.asarray(res.results[c]["out"]).astype(np.float32)
        out[:, 0, c * HC : (c + 1) * HC, :] = o.transpose(2, 0, 1, 3).reshape(
            t_total, HC, W_FULL
        )
    return out
